# revision 1
# baseline (speedup 1.0000x reference)
"""Trainium2 Bass kernel for the MU-MISO channel problem.

Math: the reference collapses algebraically to a 4x4 channel mix over the
huge [B, C] axis plus scaled noise:

    out[u, b, c] = sum_v M'[u, v] * x[v, b, c] + s'[u] * noise[u, b, c]

where  A[u, v]  = sqrt(P[v]) * sum_n H[n, u] * W[n, v]
       amp[u]   = A[u, u]
       M'       = A / amp[:, None]
       s'       = stddev / amp

M'/s' are tiny (4x4 / 4) and computed on host from W/H/P/stddev; the
O(U*B*C) streaming work runs on 8 NeuronCores, data-parallel over Batch.

Per-core layout: the per-core shard x_s[u, :] (N = 16*49152 elems) is viewed
as [U=4, Q=32, NSUP, F]; SBUF tiles are [128, F] with partition p = u*32+q.
The 4-way mix across u becomes a single 128x128 stationary matmul with
S = kron(M'.T, I_32) (block-diagonal per q), so the VectorEngine does one
fused op per element: out = (noise * s_pp) + psum.

To keep the fp32 TensorEngine (2-pass fp32 matmuls) off the critical path,
the last super-tile (1/6 of the stream) is instead computed on the
VectorEngine as a per-u scalar-chain (tiles [128, FC] per u with elementwise
alignment across u), balancing PE ~74us vs DVE ~51us, both under the ~100us
DMA wall.
"""

import sys

for _p in ("/opt/trn_rl_repo",):
    if _p not in sys.path:
        sys.path.insert(0, _p)

import numpy as np

import concourse.bass as bass
import concourse.tile as tile
from concourse import bacc, mybir
from concourse import bass_utils

# Problem shapes (hardcoded per contract)
U, NT, BATCH, CWH = 4, 8, 128, 49152
NCORES = 8
BL = BATCH // NCORES            # 16 batches per core
N = BL * CWH                    # 786432 elems per (core, u)
Q = 32                          # chunks per u -> partition p = u*32 + q
NSUP = 6                        # super-tile slots in the DRAM view
NSUP_A = 5                      # super-tiles processed via matmul layout
F = N // (Q * NSUP)             # 4096 free elems per partition per super-tile
T = 512                         # matmul free dim (one PSUM bank)
JS = F // T                     # 8 matmuls per super-tile
FC = (Q * F) // 128             # 1024: chain-tile free dim ([128, FC] per u)
FP32 = mybir.dt.float32

_CACHE = {}


def _build_program():
    """Build + compile the per-core Bass program (same program on all cores)."""
    nc = bacc.Bacc(
        "TRN2",
        target_bir_lowering=False,
        debug=False,
        enable_asserts=True,
        num_devices=NCORES,
    )
    x_d = nc.dram_tensor("x_s", [U, Q, NSUP, F], FP32, kind="ExternalInput")
    n_d = nc.dram_tensor("n_s", [U, Q, NSUP, F], FP32, kind="ExternalInput")
    S_d = nc.dram_tensor("S_mat", [128, 128], FP32, kind="ExternalInput")
    s_d = nc.dram_tensor("s_pp", [128, 1], FP32, kind="ExternalInput")
    # mp_pp[:, 4*u+v] = M'[u, v]; mp_pp[:, 16+u] = s'[u]  (broadcast over parts)
    mp_d = nc.dram_tensor("mp_pp", [128, 20], FP32, kind="ExternalInput")
    o_d = nc.dram_tensor("out_s", [U, Q, NSUP, F], FP32, kind="ExternalOutput")

    AL = mybir.AluOpType

    with tile.TileContext(nc) as tc:
        with (
            tc.tile_pool(name="const", bufs=1) as cpool,
            tc.tile_pool(name="io", bufs=3) as iopool,
            tc.tile_pool(name="chain", bufs=1) as chpool,
            tc.tile_pool(name="psum", bufs=8, space="PSUM") as pspool,
        ):
            S_t = cpool.tile([128, 128], FP32)
            nc.sync.dma_start(S_t[:], S_d[:, :])
            s_t = cpool.tile([128, 1], FP32)
            nc.sync.dma_start(s_t[:], s_d[:, :])
            mp_t = cpool.tile([128, 20], FP32)
            nc.sync.dma_start(mp_t[:], mp_d[:, :])

            F2 = F // 2
            xv_t = [None] * U
            nu_t = [None] * U

            def chain_loads():
                # last super-tile (st = NSUP-1) in per-u layout [128, FC]
                for v in range(U):
                    xv_t[v] = chpool.tile([128, FC], FP32, tag=f"xv{v}", name=f"xv{v}")
                    nc.sync.dma_start(xv_t[v][:], x_d[v, :, NSUP - 1, :])
                for u in range(U):
                    nu_t[u] = chpool.tile([128, FC], FP32, tag=f"nu{u}", name=f"nu{u}")
                    nc.sync.dma_start(nu_t[u][:], n_d[u, :, NSUP - 1, :])

            def chain_compute(u):
                ou = chpool.tile([128, FC], FP32, tag=f"ou{u}")
                nc.vector.tensor_scalar_mul(ou[:], xv_t[0][:], mp_t[:, 4 * u : 4 * u + 1])
                for v in range(1, U):
                    nc.vector.scalar_tensor_tensor(
                        out=ou[:],
                        in0=xv_t[v][:],
                        scalar=mp_t[:, 4 * u + v : 4 * u + v + 1],
                        in1=ou[:],
                        op0=AL.mult,
                        op1=AL.add,
                    )
                nc.vector.scalar_tensor_tensor(
                    out=ou[:],
                    in0=nu_t[u][:],
                    scalar=mp_t[:, 16 + u : 17 + u],
                    in1=ou[:],
                    op0=AL.mult,
                    op1=AL.add,
                )
                nc.scalar.dma_start(o_d[u, :, NSUP - 1, :], ou[:])

            for st in range(NSUP_A):
                x_t = iopool.tile([128, F], FP32, tag="x", bufs=3)
                nc.sync.dma_start(x_t[:, :F2], x_d[:, :, st, :F2])
                nc.sync.dma_start(x_t[:, F2:], x_d[:, :, st, F2:])
                n_t = iopool.tile([128, F], FP32, tag="n", bufs=3)
                nc.sync.dma_start(n_t[:, :F2], n_d[:, :, st, :F2])
                nc.sync.dma_start(n_t[:, F2:], n_d[:, :, st, F2:])
                o_t = iopool.tile([128, F], FP32, tag="o", bufs=2)
                for k in range(JS):
                    ps = pspool.tile([128, T], FP32)
                    nc.tensor.matmul(
                        ps[:],
                        S_t[:],
                        x_t[:, k * T : (k + 1) * T],
                        start=True,
                        stop=True,
                    )
                    nc.vector.scalar_tensor_tensor(
                        out=o_t[:, k * T : (k + 1) * T],
                        in0=n_t[:, k * T : (k + 1) * T],
                        scalar=s_t[:, :],
                        in1=ps[:],
                        op0=AL.mult,
                        op1=AL.add,
                    )
                if st < NSUP_A - 1:
                    nc.scalar.dma_start(o_d[:, :, st, :F2], o_t[:, :F2])
                    nc.scalar.dma_start(o_d[:, :, st, F2:], o_t[:, F2:])
                else:
                    # final super-tile: finer store splits so the stream tail
                    # drains as the last STTs finish
                    F4 = F // 4
                    for qtr in range(4):
                        nc.scalar.dma_start(
                            o_d[:, :, st, qtr * F4 : (qtr + 1) * F4],
                            o_t[:, qtr * F4 : (qtr + 1) * F4],
                        )
                if st == 1:
                    chain_loads()
                if st == 2:
                    chain_compute(0)
                    chain_compute(1)
                if st == 3:
                    chain_compute(2)
                    chain_compute(3)

    nc.compile()
    return nc


def _get_program():
    if "nc" not in _CACHE:
        _CACHE["nc"] = _build_program()
    return _CACHE["nc"]


def _host_scalars(W, H, P, stddev):
    """M' (4x4 mix), s' (noise scale) -> S_mat, s_pp, mp_pp (f32)."""
    W64 = np.asarray(W, np.float64)
    H64 = np.asarray(H, np.float64)
    P64 = np.asarray(P, np.float64)
    sd64 = np.asarray(stddev, np.float64)
    sqrtP = np.sqrt(P64)
    A = H64.T @ (W64 * sqrtP[None, :])  # A[u,v] = sum_n H[n,u] W[n,v] sqrtP[v]
    amp = np.diag(A).copy()
    Mp = A / amp[:, None]
    sp = sd64 / amp
    S_mat = np.kron(Mp.T, np.eye(Q, dtype=np.float64)).astype(np.float32)
    s_pp = np.repeat(sp, Q).astype(np.float32).reshape(128, 1)
    mp_row = np.concatenate([Mp.reshape(-1), sp]).astype(np.float32)  # [20]
    mp_pp = np.ascontiguousarray(np.broadcast_to(mp_row, (128, 20)))
    return np.ascontiguousarray(S_mat), s_pp, mp_pp


def make_in_maps(x, W, H, P, stddev, noise):
    S_mat, s_pp, mp_pp = _host_scalars(W, H, P, stddev)
    x = np.asarray(x, np.float32)
    noise = np.asarray(noise, np.float32)
    in_maps = []
    for c in range(NCORES):
        xs = np.ascontiguousarray(x[:, c * BL : (c + 1) * BL, :]).reshape(
            U, Q, NSUP, F
        )
        ns = np.ascontiguousarray(noise[:, c * BL : (c + 1) * BL, :]).reshape(
            U, Q, NSUP, F
        )
        in_maps.append(
            {"x_s": xs, "n_s": ns, "S_mat": S_mat, "s_pp": s_pp, "mp_pp": mp_pp}
        )
    return in_maps


def gather_output(results):
    out = np.empty((U, BATCH, CWH), np.float32)
    for c in range(NCORES):
        out[:, c * BL : (c + 1) * BL, :] = results[c]["out_s"].reshape(U, BL, CWH)
    return out


def run_on_hw(x, W, H, P, stddev, noise, **run_kwargs):
    nc = _get_program()
    in_maps = make_in_maps(x, W, H, P, stddev, noise)
    res = bass_utils.run_bass_kernel_spmd(
        nc, in_maps, core_ids=list(range(NCORES)), **run_kwargs
    )
    return res


def kernel(x, W, H, P, stddev, noise):
    res = run_on_hw(x, W, H, P, stddev, noise)
    return gather_output(res.results)



# revision 2
# speedup vs baseline: 1.8299x; 1.8299x over previous
"""Trainium2 Bass kernel for the MU-MISO channel problem.

Math: the reference collapses algebraically to a 4x4 channel mix over the
huge [B, C] axis plus scaled noise:

    out[u, b, c] = sum_v M'[u, v] * x[v, b, c] + s'[u] * noise[u, b, c]

where  A[u, v]  = sqrt(P[v]) * sum_n H[n, u] * W[n, v]
       amp[u]   = A[u, u]
       M'       = A / amp[:, None]
       s'       = stddev / amp

M'/s' are tiny (4x4 / 4) and computed on host from W/H/P/stddev; the
O(U*B*C) streaming work runs on 8 NeuronCores, data-parallel over Batch.

The kernel is HBM-bandwidth bound (per-NC limit ~358 GB/s; the fp32
version sits exactly at the 3*12.6 MB/core fp32 roofline ~105us). The
tolerance (rel 2e-2) leaves a large precision budget, so the stream is
quantized: x and out in bf16, noise in fp8e4 (it contributes only ~5.6%
of the output norm; fp8 quantization adds ~2e-3 rel err). Total per-core
traffic drops 37.7 MB -> 15.7 MB, i.e. a ~44us DMA roofline. Measured
end-to-end rel err ~3e-3.

Per-core layout: the per-core shard (N = 16*49152 elems per u) is viewed
as [U=4, Q=32, FLAT=24576] -> SBUF tiles are [128, Ft] with partition
p = u*32 + q. The 4-way mix across u is a single 128x128 stationary bf16
matmul with S = kron(M'.T, I_32) (block-diagonal per q); the VectorEngine
then does one fused op per element: out_bf16 = (noise_fp8 * s'_pp) + psum.
PE (~12us) and DVE (~26us) both sit well under the ~44us DMA wall.
"""

import sys

for _p in ("/opt/trn_rl_repo",):
    if _p not in sys.path:
        sys.path.insert(0, _p)

import numpy as np
import ml_dtypes

import concourse.bass as bass
import concourse.tile as tile
from concourse import bacc, mybir
from concourse import bass_utils

# Problem shapes (hardcoded per contract)
U, NT, BATCH, CWH = 4, 8, 128, 49152
NCORES = 8
BL = BATCH // NCORES            # 16 batches per core
N = BL * CWH                    # 786432 elems per (core, u)
Q = 32                          # chunks per u -> partition p = u*32 + q
FLAT = N // Q                   # 24576 free elems per partition
Ft = 4096                       # chunk free dim (8 KB bf16 per partition)
NCH = FLAT // Ft                # 6 chunks
T = 512                         # matmul free dim (one PSUM bank)
JS = Ft // T                    # 8 matmuls per chunk
FP32 = mybir.dt.float32
BF16 = mybir.dt.bfloat16
FP8 = mybir.dt.float8e4

_CACHE = {}


def _build_program():
    """Build + compile the per-core Bass program (same program on all cores)."""
    nc = bacc.Bacc(
        "TRN2",
        target_bir_lowering=False,
        debug=False,
        enable_asserts=True,
        num_devices=NCORES,
    )
    x_d = nc.dram_tensor("x_s", [128, FLAT], BF16, kind="ExternalInput")
    n_d = nc.dram_tensor("n_s", [128, FLAT], FP8, kind="ExternalInput")
    S_d = nc.dram_tensor("S_mat", [128, 128], BF16, kind="ExternalInput")
    s_d = nc.dram_tensor("s_pp", [128, 1], FP32, kind="ExternalInput")
    o_d = nc.dram_tensor("out_s", [128, FLAT], BF16, kind="ExternalOutput")

    AL = mybir.AluOpType

    with tile.TileContext(nc) as tc:
        with (
            tc.tile_pool(name="const", bufs=1) as cpool,
            tc.tile_pool(name="io", bufs=3) as iopool,
            tc.tile_pool(name="psum", bufs=8, space="PSUM") as pspool,
        ):
            S_t = cpool.tile([128, 128], BF16)
            nc.sync.dma_start(S_t[:], S_d[:, :])
            s_t = cpool.tile([128, 1], FP32)
            nc.sync.dma_start(s_t[:], s_d[:, :])

            for ch in range(NCH):
                lo = ch * Ft
                hi = lo + Ft
                x_t = iopool.tile([128, Ft], BF16, tag="x", bufs=3)
                nc.sync.dma_start(x_t[:], x_d[:, lo:hi])
                n_t = iopool.tile([128, Ft], FP8, tag="n", bufs=3)
                nc.sync.dma_start(n_t[:], n_d[:, lo:hi])
                o_t = iopool.tile([128, Ft], BF16, tag="o", bufs=3)
                for k in range(JS):
                    ps = pspool.tile([128, T], FP32)
                    nc.tensor.matmul(
                        ps[:],
                        S_t[:],
                        x_t[:, k * T : (k + 1) * T],
                        start=True,
                        stop=True,
                    )
                    nc.vector.scalar_tensor_tensor(
                        out=o_t[:, k * T : (k + 1) * T],
                        in0=n_t[:, k * T : (k + 1) * T],
                        scalar=s_t[:, :],
                        in1=ps[:],
                        op0=AL.mult,
                        op1=AL.add,
                    )
                if ch < NCH - 1:
                    nc.scalar.dma_start(o_d[:, lo:hi], o_t[:])
                else:
                    # final chunk: finer store splits so the stream tail
                    # drains as the last combines finish
                    F4 = Ft // 4
                    for qtr in range(4):
                        nc.scalar.dma_start(
                            o_d[:, lo + qtr * F4 : lo + (qtr + 1) * F4],
                            o_t[:, qtr * F4 : (qtr + 1) * F4],
                        )

    nc.compile()
    return nc


def _get_program():
    if "nc" not in _CACHE:
        _CACHE["nc"] = _build_program()
    return _CACHE["nc"]


def _host_scalars(W, H, P, stddev):
    """M' (4x4 mix), s' (noise scale) -> S_mat (bf16), s_pp (f32)."""
    W64 = np.asarray(W, np.float64)
    H64 = np.asarray(H, np.float64)
    P64 = np.asarray(P, np.float64)
    sd64 = np.asarray(stddev, np.float64)
    sqrtP = np.sqrt(P64)
    A = H64.T @ (W64 * sqrtP[None, :])  # A[u,v] = sum_n H[n,u] W[n,v] sqrtP[v]
    amp = np.diag(A).copy()
    Mp = A / amp[:, None]
    sp = sd64 / amp
    S_mat = np.kron(Mp.T, np.eye(Q, dtype=np.float64)).astype(ml_dtypes.bfloat16)
    s_pp = np.repeat(sp, Q).astype(np.float32).reshape(128, 1)
    return np.ascontiguousarray(S_mat), s_pp


def make_in_maps(x, W, H, P, stddev, noise):
    S_mat, s_pp = _host_scalars(W, H, P, stddev)
    x16 = np.asarray(x, np.float32).astype(ml_dtypes.bfloat16)
    n8 = np.asarray(noise, np.float32).astype(ml_dtypes.float8_e4m3)
    in_maps = []
    for c in range(NCORES):
        xs = np.ascontiguousarray(x16[:, c * BL : (c + 1) * BL, :]).reshape(128, FLAT)
        ns = np.ascontiguousarray(n8[:, c * BL : (c + 1) * BL, :]).reshape(128, FLAT)
        in_maps.append({"x_s": xs, "n_s": ns, "S_mat": S_mat, "s_pp": s_pp})
    return in_maps


def gather_output(results):
    out = np.empty((U, BATCH, CWH), np.float32)
    for c in range(NCORES):
        out[:, c * BL : (c + 1) * BL, :] = (
            results[c]["out_s"].reshape(U, BL, CWH).astype(np.float32)
        )
    return out


def run_on_hw(x, W, H, P, stddev, noise, **run_kwargs):
    nc = _get_program()
    in_maps = make_in_maps(x, W, H, P, stddev, noise)
    res = bass_utils.run_bass_kernel_spmd(
        nc, in_maps, core_ids=list(range(NCORES)), **run_kwargs
    )
    return res


def kernel(x, W, H, P, stddev, noise):
    res = run_on_hw(x, W, H, P, stddev, noise)
    return gather_output(res.results)


# revision 3
# speedup vs baseline: 1.9333x; 1.0565x over previous
"""Trainium2 Bass kernel for the MU-MISO channel problem.

Math: the reference collapses algebraically to a 4x4 channel mix over the
huge [B, C] axis plus scaled noise:

    out[u, b, c] = sum_v M'[u, v] * x[v, b, c] + s'[u] * noise[u, b, c]

where  A[u, v]  = sqrt(P[v]) * sum_n H[n, u] * W[n, v]
       amp[u]   = A[u, u]
       M'       = A / amp[:, None]
       s'       = stddev / amp

M'/s' are tiny (4x4 / 4) and computed on host from W/H/P/stddev; the
O(U*B*C) streaming work runs on 8 NeuronCores, data-parallel over Batch.

The kernel is HBM-bandwidth bound (per-NC limit ~358 GB/s; the fp32
version sits exactly at the 3*12.6 MB/core fp32 roofline ~105us). The
tolerance (rel 2e-2) leaves a large precision budget, so the stream is
quantized: x and out in bf16, noise in fp8e4 (it contributes only ~5.6%
of the output norm; fp8 quantization adds ~2e-3 rel err). Total per-core
traffic drops 37.7 MB -> 15.7 MB, i.e. a ~44us DMA roofline. Measured
end-to-end rel err ~3e-3.

Per-core layout: the per-core shard (N = 16*49152 elems per u) is viewed
as [U=4, Q=32, FLAT=24576] -> SBUF tiles are [128, Ft] with partition
p = u*32 + q. The 4-way mix across u is a single 128x128 stationary bf16
matmul with S = kron(M'.T, I_32) (block-diagonal per q); the VectorEngine
then does one fused op per element: out_bf16 = (noise_fp8 * s'_pp) + psum.
PE (~12us) and DVE (~26us) both sit well under the ~44us DMA wall.
"""

import sys

for _p in ("/opt/trn_rl_repo",):
    if _p not in sys.path:
        sys.path.insert(0, _p)

import numpy as np
import ml_dtypes

import concourse.bass as bass
import concourse.tile as tile
from concourse import bacc, mybir
from concourse import bass_utils

# Problem shapes (hardcoded per contract)
U, NT, BATCH, CWH = 4, 8, 128, 49152
NCORES = 8
BL = BATCH // NCORES            # 16 batches per core
N = BL * CWH                    # 786432 elems per (core, u)
Q = 32                          # chunks per u -> partition p = u*32 + q
FLAT = N // Q                   # 24576 free elems per partition
Ft = 4096                       # chunk free dim (8 KB bf16 per partition)
NCH = FLAT // Ft                # 6 chunks
T = 512                         # matmul free dim (one PSUM bank)
JS = Ft // T                    # 8 matmuls per chunk
FP32 = mybir.dt.float32
BF16 = mybir.dt.bfloat16
FP8 = mybir.dt.float8e4

_CACHE = {}


def _build_program():
    """Build + compile the per-core Bass program (same program on all cores)."""
    nc = bacc.Bacc(
        "TRN2",
        target_bir_lowering=False,
        debug=False,
        enable_asserts=True,
        num_devices=NCORES,
    )
    x_d = nc.dram_tensor("x_s", [128, FLAT], BF16, kind="ExternalInput")
    n_d = nc.dram_tensor("n_s", [128, FLAT], FP8, kind="ExternalInput")
    S_d = nc.dram_tensor("S_mat", [128, 128], BF16, kind="ExternalInput")
    s_d = nc.dram_tensor("s_pp", [128, 1], FP32, kind="ExternalInput")
    o_d = nc.dram_tensor("out_s", [128, FLAT], BF16, kind="ExternalOutput")

    AL = mybir.AluOpType

    HF = Ft // 2  # half-chunk: 4 PSUM banks / one STT / one store split

    with tile.TileContext(nc) as tc:
        with (
            tc.tile_pool(name="const", bufs=1) as cpool,
            tc.tile_pool(name="io", bufs=3) as iopool,
            tc.tile_pool(name="psum", bufs=2, space="PSUM") as pspool,
        ):
            # constants go on the scalar (store) queue: tiny transfers at the
            # head of the sync queue would delay the first 0.5 MB x load by
            # their ~2us completion latency each
            S_t = cpool.tile([128, 128], BF16)
            nc.scalar.dma_start(S_t[:], S_d[:, :])
            s_t = cpool.tile([128, 1], FP32)
            nc.scalar.dma_start(s_t[:], s_d[:, :])

            for ch in range(NCH):
                lo = ch * Ft
                x_t = iopool.tile([128, Ft], BF16, tag="x", bufs=3)
                nc.sync.dma_start(x_t[:, :HF], x_d[:, lo : lo + HF])
                nc.sync.dma_start(x_t[:, HF:], x_d[:, lo + HF : lo + Ft])
                n_t = iopool.tile([128, Ft], FP8, tag="n", bufs=3)
                nc.sync.dma_start(n_t[:], n_d[:, lo : lo + Ft])
                o_t = iopool.tile([128, Ft], BF16, tag="o", bufs=3)
                for half in range(2):
                    # one [128, HF] PSUM tile = 4 banks; 4 matmuls fill it,
                    # then a single wide STT drains it (amortizes the ~195ns
                    # per-instruction PSUM-latency + decode overhead 4x)
                    ps = pspool.tile([128, HF], FP32)
                    hlo = half * HF
                    for k in range(HF // T):
                        nc.tensor.matmul(
                            ps[:, k * T : (k + 1) * T],
                            S_t[:],
                            x_t[:, hlo + k * T : hlo + (k + 1) * T],
                            start=True,
                            stop=True,
                        )
                    nc.vector.scalar_tensor_tensor(
                        out=o_t[:, hlo : hlo + HF],
                        in0=n_t[:, hlo : hlo + HF],
                        scalar=s_t[:, :],
                        in1=ps[:],
                        op0=AL.mult,
                        op1=AL.add,
                    )
                    if ch < NCH - 1:
                        nc.scalar.dma_start(
                            o_d[:, lo + hlo : lo + hlo + HF], o_t[:, hlo : hlo + HF]
                        )
                    else:
                        # final chunk: finer store splits so the stream tail
                        # drains as the last combines finish
                        F4 = HF // 2
                        for qtr in range(2):
                            nc.scalar.dma_start(
                                o_d[
                                    :,
                                    lo + hlo + qtr * F4 : lo + hlo + (qtr + 1) * F4,
                                ],
                                o_t[:, hlo + qtr * F4 : hlo + (qtr + 1) * F4],
                            )

    nc.compile()
    return nc


def _get_program():
    if "nc" not in _CACHE:
        _CACHE["nc"] = _build_program()
    return _CACHE["nc"]


def _host_scalars(W, H, P, stddev):
    """M' (4x4 mix), s' (noise scale) -> S_mat (bf16), s_pp (f32)."""
    W64 = np.asarray(W, np.float64)
    H64 = np.asarray(H, np.float64)
    P64 = np.asarray(P, np.float64)
    sd64 = np.asarray(stddev, np.float64)
    sqrtP = np.sqrt(P64)
    A = H64.T @ (W64 * sqrtP[None, :])  # A[u,v] = sum_n H[n,u] W[n,v] sqrtP[v]
    amp = np.diag(A).copy()
    Mp = A / amp[:, None]
    sp = sd64 / amp
    S_mat = np.kron(Mp.T, np.eye(Q, dtype=np.float64)).astype(ml_dtypes.bfloat16)
    s_pp = np.repeat(sp, Q).astype(np.float32).reshape(128, 1)
    return np.ascontiguousarray(S_mat), s_pp


def make_in_maps(x, W, H, P, stddev, noise):
    S_mat, s_pp = _host_scalars(W, H, P, stddev)
    x16 = np.asarray(x, np.float32).astype(ml_dtypes.bfloat16)
    n8 = np.asarray(noise, np.float32).astype(ml_dtypes.float8_e4m3)
    in_maps = []
    for c in range(NCORES):
        xs = np.ascontiguousarray(x16[:, c * BL : (c + 1) * BL, :]).reshape(128, FLAT)
        ns = np.ascontiguousarray(n8[:, c * BL : (c + 1) * BL, :]).reshape(128, FLAT)
        in_maps.append({"x_s": xs, "n_s": ns, "S_mat": S_mat, "s_pp": s_pp})
    return in_maps


def gather_output(results):
    out = np.empty((U, BATCH, CWH), np.float32)
    for c in range(NCORES):
        out[:, c * BL : (c + 1) * BL, :] = (
            results[c]["out_s"].reshape(U, BL, CWH).astype(np.float32)
        )
    return out


def run_on_hw(x, W, H, P, stddev, noise, **run_kwargs):
    nc = _get_program()
    in_maps = make_in_maps(x, W, H, P, stddev, noise)
    res = bass_utils.run_bass_kernel_spmd(
        nc, in_maps, core_ids=list(range(NCORES)), **run_kwargs
    )
    return res


def kernel(x, W, H, P, stddev, noise):
    res = run_on_hw(x, W, H, P, stddev, noise)
    return gather_output(res.results)


# revision 4
# speedup vs baseline: 2.2402x; 1.1587x over previous
"""Trainium2 Bass kernel for the MU-MISO channel problem.

Math: the reference collapses algebraically to a 4x4 channel mix over the
huge [B, C] axis plus scaled noise:

    out[u, b, c] = sum_v M'[u, v] * x[v, b, c] + s'[u] * noise[u, b, c]

where  A[u, v]  = sqrt(P[v]) * sum_n H[n, u] * W[n, v]
       amp[u]   = A[u, u]
       M'       = A / amp[:, None]
       s'       = stddev / amp

M'/s' are tiny (4x4 / 4) and computed on host from W/H/P/stddev; the
O(U*B*C) streaming work runs on 8 NeuronCores, data-parallel over Batch.

The kernel is HBM-bandwidth bound (per-NC limit ~358 GB/s; the fp32
version sits exactly at the 3*12.6 MB/core fp32 roofline ~105us). The
tolerance (rel 2e-2) leaves a large precision budget, so the stream is
quantized: x and out in bf16, noise in fp8e4 (it contributes only ~5.6%
of the output norm; fp8 quantization adds ~2e-3 rel err). Total per-core
traffic drops 37.7 MB -> 15.7 MB, i.e. a ~44us DMA roofline. Measured
end-to-end rel err ~3e-3.

Per-core layout: the per-core shard (N = 16*49152 elems per u) is viewed
as [U=4, Q=32, FLAT=24576] -> SBUF tiles are [128, Ft] with partition
p = u*32 + q. The 4-way mix across u is a single 128x128 stationary bf16
matmul with S = kron(M'.T, I_32) (block-diagonal per q); the VectorEngine
then does one fused op per element: out_bf16 = (noise_fp8 * s'_pp) + psum.
PE (~12us) and DVE (~26us) both sit well under the ~44us DMA wall.
"""

import sys

for _p in ("/opt/trn_rl_repo",):
    if _p not in sys.path:
        sys.path.insert(0, _p)

import numpy as np
import ml_dtypes

import concourse.bass as bass
import concourse.tile as tile
from concourse import bacc, mybir
from concourse import bass_utils

# Problem shapes (hardcoded per contract)
U, NT, BATCH, CWH = 4, 8, 128, 49152
NCORES = 8
BL = BATCH // NCORES            # 16 batches per core
N = BL * CWH                    # 786432 elems per (core, u)
Q = 32                          # chunks per u -> partition p = u*32 + q
FLAT = N // Q                   # 24576 free elems per partition
Ft = 4096                       # chunk free dim (8 KB bf16 per partition)
NCH = FLAT // Ft                # 6 chunks
T = 512                         # matmul free dim (one PSUM bank)
JS = Ft // T                    # 8 matmuls per chunk
FP32 = mybir.dt.float32
BF16 = mybir.dt.bfloat16
FP8 = mybir.dt.float8e4
FP8X = mybir.dt.float8e3

_CACHE = {}


def _build_program():
    """Build + compile the per-core Bass program (same program on all cores)."""
    nc = bacc.Bacc(
        "TRN2",
        target_bir_lowering=False,
        debug=False,
        enable_asserts=True,
        num_devices=NCORES,
    )
    x_d = nc.dram_tensor("x_s", [128, FLAT], FP8X, kind="ExternalInput")
    n_d = nc.dram_tensor("n_s", [128, FLAT], FP8, kind="ExternalInput")
    S_d = nc.dram_tensor("S_mat", [128, 128], BF16, kind="ExternalInput")
    s_d = nc.dram_tensor("s_pp", [128, 1], FP32, kind="ExternalInput")
    o_d = nc.dram_tensor("out_s", [128, FLAT], BF16, kind="ExternalOutput")

    AL = mybir.AluOpType

    HF = Ft // 2  # half-chunk: 4 PSUM banks / one STT / one store split

    with tile.TileContext(nc) as tc:
        with (
            tc.tile_pool(name="const", bufs=1) as cpool,
            tc.tile_pool(name="io", bufs=3) as iopool,
            tc.tile_pool(name="psum", bufs=2, space="PSUM") as pspool,
        ):
            # constants go on the scalar (store) queue: tiny transfers at the
            # head of the sync queue would delay the first 0.5 MB x load by
            # their ~2us completion latency each
            S_t = cpool.tile([128, 128], BF16)
            nc.scalar.dma_start(S_t[:], S_d[:, :])
            s_t = cpool.tile([128, 1], FP32)
            nc.scalar.dma_start(s_t[:], s_d[:, :])

            for ch in range(NCH):
                lo = ch * Ft
                x_t = iopool.tile([128, Ft], FP8X, tag="x", bufs=4)
                nc.sync.dma_start(x_t[:, :HF], x_d[:, lo : lo + HF])
                nc.sync.dma_start(x_t[:, HF:], x_d[:, lo + HF : lo + Ft])
                n_t = iopool.tile([128, Ft], FP8, tag="n", bufs=4)
                nc.sync.dma_start(n_t[:], n_d[:, lo : lo + Ft])
                o_t = iopool.tile([128, Ft], BF16, tag="o", bufs=3)
                for half in range(2):
                    # one [128, HF] PSUM tile = 4 banks; 4 matmuls fill it,
                    # then a single wide STT drains it (amortizes the ~195ns
                    # per-instruction PSUM-latency + decode overhead 4x)
                    ps = pspool.tile([128, HF], FP32)
                    hlo = half * HF
                    for k in range(HF // T):
                        nc.tensor.matmul(
                            ps[:, k * T : (k + 1) * T],
                            S_t[:],
                            x_t[:, hlo + k * T : hlo + (k + 1) * T],
                            start=True,
                            stop=True,
                        )
                    nc.vector.scalar_tensor_tensor(
                        out=o_t[:, hlo : hlo + HF],
                        in0=n_t[:, hlo : hlo + HF],
                        scalar=s_t[:, :],
                        in1=ps[:],
                        op0=AL.mult,
                        op1=AL.add,
                    )
                    if ch < NCH - 1:
                        nc.scalar.dma_start(
                            o_d[:, lo + hlo : lo + hlo + HF], o_t[:, hlo : hlo + HF]
                        )
                    else:
                        # final chunk: finer store splits so the stream tail
                        # drains as the last combines finish
                        F4 = HF // 2
                        for qtr in range(2):
                            nc.scalar.dma_start(
                                o_d[
                                    :,
                                    lo + hlo + qtr * F4 : lo + hlo + (qtr + 1) * F4,
                                ],
                                o_t[:, hlo + qtr * F4 : hlo + (qtr + 1) * F4],
                            )

    nc.compile()
    return nc


def _get_program():
    if "nc" not in _CACHE:
        _CACHE["nc"] = _build_program()
    return _CACHE["nc"]


def _host_scalars(W, H, P, stddev):
    """M' (4x4 mix), s' (noise scale) -> S_mat (bf16), s_pp (f32)."""
    W64 = np.asarray(W, np.float64)
    H64 = np.asarray(H, np.float64)
    P64 = np.asarray(P, np.float64)
    sd64 = np.asarray(stddev, np.float64)
    sqrtP = np.sqrt(P64)
    A = H64.T @ (W64 * sqrtP[None, :])  # A[u,v] = sum_n H[n,u] W[n,v] sqrtP[v]
    amp = np.diag(A).copy()
    Mp = A / amp[:, None]
    sp = sd64 / amp
    S_mat = np.kron(Mp.T, np.eye(Q, dtype=np.float64)).astype(ml_dtypes.bfloat16)
    s_pp = np.repeat(sp, Q).astype(np.float32).reshape(128, 1)
    return np.ascontiguousarray(S_mat), s_pp


def make_in_maps(x, W, H, P, stddev, noise):
    S_mat, s_pp = _host_scalars(W, H, P, stddev)
    x16 = np.asarray(x, np.float32).astype(ml_dtypes.float8_e3m4)
    n8 = np.asarray(noise, np.float32).astype(ml_dtypes.float8_e4m3)
    in_maps = []
    for c in range(NCORES):
        xs = np.ascontiguousarray(x16[:, c * BL : (c + 1) * BL, :]).reshape(128, FLAT)
        ns = np.ascontiguousarray(n8[:, c * BL : (c + 1) * BL, :]).reshape(128, FLAT)
        in_maps.append({"x_s": xs, "n_s": ns, "S_mat": S_mat, "s_pp": s_pp})
    return in_maps


def gather_output(results):
    out = np.empty((U, BATCH, CWH), np.float32)
    for c in range(NCORES):
        out[:, c * BL : (c + 1) * BL, :] = (
            results[c]["out_s"].reshape(U, BL, CWH).astype(np.float32)
        )
    return out


def run_on_hw(x, W, H, P, stddev, noise, **run_kwargs):
    nc = _get_program()
    in_maps = make_in_maps(x, W, H, P, stddev, noise)
    res = bass_utils.run_bass_kernel_spmd(
        nc, in_maps, core_ids=list(range(NCORES)), **run_kwargs
    )
    return res


def kernel(x, W, H, P, stddev, noise):
    res = run_on_hw(x, W, H, P, stddev, noise)
    return gather_output(res.results)
